# revision 1
# baseline (speedup 1.0000x reference)
"""Multi-head attention block for Trainium2, 8-core data-parallel SPMD.

Computes, per batch element b (one NeuronCore each):
    qkv = x @ w_qkv ; q,k,v split into 16 heads of dim 64
    attn = softmax(q @ k^T / sqrt(64)) ; out = (attn @ v) @ w_out + b_out

Strategy (per core):
  - transpose x -> xT (c-major) via PE transposes
  - v computed in natural layout, written strided into v_aug tiles with a
    ones-column per head so the attention output matmul also produces the
    softmax row-sums for free
  - attention per head in transposed layout: s^T = kT^T @ qT on the PE,
    exp on ACT (1/8 scale folded in), o^T_aug accumulated over k chunks;
    softmax normalization deferred to o^T (DVE reciprocal + K=1 ones-matmul
    partition-broadcast)
  - the qT/kT projection of head pair t+1 is explicitly interleaved into
    the attention instruction stream of pair t (engines execute their
    streams in order, so overlap has to be emitted, not just scheduled)
  - out = o^T^T @ w_out + ones x b_out (bias added by the PE)
All matmul-feeding tiles are declared float32r (full PE rate; the producing
DVE/ACT/DMA instructions emit the FP32r rounding the BIR verifier requires).
"""

import sys

if "/opt/trn_rl_repo" not in sys.path:
    sys.path.insert(0, "/opt/trn_rl_repo")

import numpy as np

B = 8
N = 1024  # sequence length
C = 1024  # model dim
H = 16  # heads
D = 64  # head dim
P = 128  # partitions
NT = N // P  # seq chunks
CT = C // P  # channel chunks
HP = H // 2  # head pairs
SCALE = D ** -0.5
HF = C // 512  # free-dim halves per 1024 row

_CACHE = {}


def _build_program():
    from concourse import bacc, mybir
    import concourse.tile as tile
    from concourse.masks import make_identity

    f32 = mybir.dt.float32
    f32r = mybir.dt.float32r
    Exp = mybir.ActivationFunctionType.Exp

    nc = bacc.Bacc("TRN2", target_bir_lowering=False, debug=False)
    x_d = nc.declare_dram_parameter("x", [N, C], f32r, isOutput=False)
    wqkv_d = nc.declare_dram_parameter("w_qkv", [C, 3 * C], f32r, isOutput=False)
    wout_d = nc.declare_dram_parameter("w_out", [C, C], f32r, isOutput=False)
    bout_d = nc.declare_dram_parameter("b_out", [1, C], f32r, isOutput=False)
    out_d = nc.declare_dram_parameter("out", [N, C], f32, isOutput=True)

    with tile.TileContext(nc) as tc:
        with (
            tc.tile_pool(name="consts", bufs=1) as consts,
            tc.tile_pool(name="xTo", bufs=CT) as xT_pool,
            tc.tile_pool(name="vaug", bufs=NT) as vaug_pool,
            tc.tile_pool(name="psum", bufs=1, space="PSUM") as psum,
            tc.tile_pool(name="oTp", bufs=CT) as oT_pool,
            tc.tile_pool(name="io", bufs=3) as io_pool,
            tc.tile_pool(name="w", bufs=CT) as w_pool,
            tc.tile_pool(name="wqk", bufs=4) as wqk_pool,
            tc.tile_pool(name="pT", bufs=8) as pT_pool,
            tc.tile_pool(name="recip", bufs=1) as recip_pool,
            tc.tile_pool(name="bcs", bufs=1) as bcs_pool,
            tc.tile_pool(name="qkT", bufs=4) as qkT_pool,
        ):
            identity_f32 = consts.tile(
                [P, P], f32, name="identity_f32", tag="identity_f32"
            )
            make_identity(nc, identity_f32)
            # f32r transpose runs 1.5 PE cycles/row vs 2.0 for f32
            identity = consts.tile([P, P], f32r, name="identity", tag="identity")
            nc.vector.tensor_copy(identity[:, :], identity_f32[:, :])
            # memset can't emit f32r (ISA check) — stage in f32, round via copy
            ones_f32 = consts.tile([P, P], f32, name="ones_f32", tag="ones_f32")
            nc.vector.memset(ones_f32, 1.0)
            ones = consts.tile([1, P], f32r, name="ones", tag="ones")
            nc.vector.tensor_copy(ones[0:1, :], ones_f32[0:1, :])
            b_row = consts.tile([1, C], f32r, name="b_row", tag="b_row")
            nc.sync.dma_start(out=b_row[0:1, :], in_=bout_d[0:1, :])

            xT = [
                xT_pool.tile([P, N], f32r, name=f"xT{i}", tag="xTo") for i in range(CT)
            ]
            vaug = [
                vaug_pool.tile([P, H * (D + 1)], f32r, name=f"vaug{i}", tag="vaug")
                for i in range(NT)
            ]

            def mm_tile(name, tag, bufs):
                return psum.tile([P, C], f32, name=name, tag=tag, bufs=bufs)

            def half_tile(name, tag, bufs):
                return psum.tile([P, 512], f32, name=name, tag=tag, bufs=bufs)

            # ---------------- phase 0: transpose x into xT ----------------
            for si in range(NT):
                xin = io_pool.tile([P, C], f32r, name=f"xin{si}", tag="io")
                nc.sync.dma_start(out=xin[:, :], in_=x_d[si * P : (si + 1) * P, :])
                tr_ps = psum.tile([P, C], f32r, name=f"tr{si}", tag="mm", bufs=2)
                for ci in range(CT):
                    nc.tensor.transpose(
                        tr_ps[:, ci * P : (ci + 1) * P],
                        xin[:, ci * P : (ci + 1) * P],
                        identity,
                    )
                for ci in range(CT):
                    nc.vector.tensor_copy(
                        xT[ci][:, si * P : (si + 1) * P],
                        tr_ps[:, ci * P : (ci + 1) * P],
                    )

            # ---------- phase 1B: v (natural layout) -> v_aug ----------
            # 4 seq-chunks per pass (2x [P,C] from mm/acc tags + 2 halves in
            # the sT slots) -> w_v rows streamed only twice.
            for sc0 in range(0, NT, 4):
                scs = list(range(sc0, sc0 + 4))
                full = {scs[0]: mm_tile(f"vps{scs[0]}", "mm", 2),
                        scs[1]: mm_tile(f"vps{scs[1]}", "mm", 2),
                        scs[2]: mm_tile(f"vps{scs[2]}", "acc", 1)}
                sc3 = scs[3]
                halves = [
                    half_tile(f"vps{sc3}_0", "sT", 2),
                    half_tile(f"vps{sc3}_1", "sT", 2),
                ]
                for ci in range(CT):
                    wv = w_pool.tile([P, C], f32r, name=f"wv{sc0}_{ci}", tag="w")
                    nc.sync.dma_start(
                        out=wv[:, :],
                        in_=wqkv_d[ci * P : (ci + 1) * P, 2 * C : 3 * C],
                    )
                    st = dict(start=(ci == 0), stop=(ci == CT - 1))
                    for hf in range(HF):
                        sl = slice(hf * 512, hf * 512 + 512)
                        for sc in scs[:3]:
                            nc.tensor.matmul(
                                full[sc][:, sl],
                                xT[ci][:, sc * P : (sc + 1) * P],
                                wv[:, sl],
                                **st,
                            )
                        nc.tensor.matmul(
                            halves[hf][:, :],
                            xT[ci][:, sc3 * P : (sc3 + 1) * P],
                            wv[:, sl],
                            **st,
                        )
                for sc in scs[:3]:
                    va3 = vaug[sc].rearrange("p (h u) -> p h u", u=D + 1)
                    nc.vector.tensor_copy(
                        va3[:, :, D : D + 1],
                        ones_f32[:, 0:H].rearrange("p (h u) -> p h u", u=1),
                    )
                    nc.vector.tensor_copy(
                        va3[:, :, 0:D],
                        full[sc].rearrange("p (h u) -> p h u", u=D),
                    )
                va3 = vaug[sc3].rearrange("p (h u) -> p h u", u=D + 1)
                nc.vector.tensor_copy(
                    va3[:, :, D : D + 1],
                    ones_f32[:, 0:H].rearrange("p (h u) -> p h u", u=1),
                )
                for hf in range(HF):
                    nc.vector.tensor_copy(
                        va3[:, 8 * hf : 8 * hf + 8, 0:D],
                        halves[hf].rearrange("p (h u) -> p h u", u=D),
                    )

            # ---- interleaved: attention pair t || qT/kT projection pair t+1 ----
            def qkv_pair_steps(t, qTt, kTt, q_ps, k_ps):
                """Generator: one ci-step (2 weight DMAs + 4 matmuls) per next();
                finishes with the PSUM->SBUF copies."""
                for ci in range(CT):
                    wq = wqk_pool.tile([P, P], f32r, name=f"wq{t}_{ci}", tag="wqk")
                    nc.sync.dma_start(
                        out=wq[:, :],
                        in_=wqkv_d[ci * P : (ci + 1) * P, t * P : (t + 1) * P],
                    )
                    wk = wqk_pool.tile([P, P], f32r, name=f"wk{t}_{ci}", tag="wqk")
                    nc.sync.dma_start(
                        out=wk[:, :],
                        in_=wqkv_d[ci * P : (ci + 1) * P, C + t * P : C + (t + 1) * P],
                    )
                    st = dict(start=(ci == 0), stop=(ci == CT - 1))
                    for hf in range(HF):
                        sl = slice(hf * 512, hf * 512 + 512)
                        nc.tensor.matmul(q_ps[:, sl], wq[:, :], xT[ci][:, sl], **st)
                        nc.tensor.matmul(k_ps[:, sl], wk[:, :], xT[ci][:, sl], **st)
                    yield
                nc.vector.tensor_copy(qTt[:, :], q_ps[:, :])
                nc.vector.tensor_copy(kTt[:, :], k_ps[:, :])
                yield

            def new_pair_qkv(t):
                qTt = qkT_pool.tile([P, N], f32r, name=f"qT{t}", tag="qkT")
                kTt = qkT_pool.tile([P, N], f32r, name=f"kT{t}", tag="qkT")
                q_ps = mm_tile(f"qps{t}", "mm", 2)
                k_ps = mm_tile(f"kps{t}", "mm", 2)
                return qTt, kTt, qkv_pair_steps(t, qTt, kTt, q_ps, k_ps)

            oT = [
                oT_pool.tile([P, N], f32r, name=f"oT{i}", tag="oTp")
                for i in range(CT)
            ]

            # prologue: pair 0 projection emitted straight
            qT_cur, kT_cur, gen0 = new_pair_qkv(0)
            for _ in gen0:
                pass

            # w_out is prefetched one row-chunk per head pair (inside the
            # pair loop) so the DMAs spread across the attention region
            wos = []

            def prefetch_wo(ci):
                wo = w_pool.tile([P, C], f32r, name=f"wo{ci}", tag="w")
                nc.sync.dma_start(out=wo[:, :], in_=wout_d[ci * P : (ci + 1) * P, :])
                wos.append(wo)

            pending_norm = None
            for t in range(HP):
                prefetch_wo(t)
                if t + 1 < HP:
                    qT_nxt, kT_nxt, gen = new_pair_qkv(t + 1)
                else:
                    qT_nxt = kT_nxt = gen = None
                chunk_idx = 0
                NCH = NT * HF  # 16 chunks per head
                LAG = 4  # o^T matmuls trail s/exp by LAG chunks so the
                # previous head's normalize chain hides inside the stream
                for j in range(2):
                    h = 2 * t + j
                    row0 = D * j
                    acc = mm_tile(f"acc{h}", "acc", 1)

                    def ot_mm(c, acc=acc, h=h):
                        kc, hf = divmod(c, HF)
                        sl = slice(hf * 512, hf * 512 + 512)
                        nc.tensor.matmul(
                            acc[0 : D + 1, sl],
                            vaug[kc][:, h * (D + 1) : (h + 1) * (D + 1)],
                            pts[c][:, :],
                            start=(kc == 0),
                            stop=(kc == NT - 1),
                        )

                    pts = {}
                    for c in range(NCH):
                        kc, hf = divmod(c, HF)
                        sl = slice(hf * 512, hf * 512 + 512)
                        s_ps = half_tile(f"s{h}_{kc}_{hf}", "sT", 2)
                        nc.tensor.matmul(
                            s_ps[:, :],
                            kT_cur[row0 : row0 + D, kc * P : (kc + 1) * P],
                            qT_cur[row0 : row0 + D, sl],
                            start=True,
                            stop=True,
                        )
                        pt = pT_pool.tile(
                            [P, 512], f32r, name=f"pt{h}_{kc}_{hf}", tag="pT"
                        )
                        nc.scalar.activation(
                            out=pt[:, :], in_=s_ps[:, :], func=Exp, scale=SCALE
                        )
                        pts[c] = pt
                        if c == LAG - 2 and pending_norm is not None:
                            pending_norm()
                            pending_norm = None
                        if c >= LAG:
                            ot_mm(c - LAG)
                            del pts[c - LAG]
                        # sprinkle next pair's projection into the stream
                        if gen is not None and chunk_idx % 3 == 2:
                            next(gen, None)
                        chunk_idx += 1
                    for c in range(NCH - LAG, NCH):
                        ot_mm(c)

                    def normalize(h=h, row0=row0, t=t, acc=acc):
                        # o^T[d, q] *= 1 / rowsum[q]
                        rc = recip_pool.tile([1, N], f32r, name=f"rc{h}", tag="recip")
                        with nc.allow_low_precision(
                            reason="softmax norm reciprocal rounded to f32r "
                            "for the PE broadcast matmul"
                        ):
                            nc.vector.reciprocal(rc[0:1, :], acc[D : D + 1, :])
                        bcs = bcs_pool.tile([D, N], f32, name=f"bcs{h}", tag="bcs")
                        for hf in range(HF):
                            sl = slice(hf * 512, hf * 512 + 512)
                            bc = half_tile(f"bc{h}_{hf}", "sT", 2)
                            nc.tensor.matmul(
                                bc[0:D, :],
                                ones[0:1, 0:D],
                                rc[0:1, sl],
                                start=True,
                                stop=True,
                            )
                            # DVE reads at most one PSUM operand: stage in SBUF
                            nc.vector.tensor_copy(bcs[0:D, sl], bc[0:D, :])
                        nc.vector.tensor_mul(
                            oT[t][row0 : row0 + D, :],
                            acc[0:D, :],
                            bcs[0:D, :],
                        )

                    pending_norm = normalize
                if gen is not None:
                    for _ in gen:
                        pass
                qT_cur, kT_cur = qT_nxt, kT_nxt
            pending_norm()  # last head's normalize

            # ---------------- phase 3: out = o @ w_out + b ----------------
            for sc in range(NT):
                o_ps = mm_tile(f"ops{sc}", "mm", 2)
                for ci in range(CT):
                    for hf in range(HF):
                        sl = slice(hf * 512, hf * 512 + 512)
                        nc.tensor.matmul(
                            o_ps[:, sl],
                            oT[ci][:, sc * P : (sc + 1) * P],
                            wos[ci][:, sl],
                            start=(ci == 0),
                            stop=False,
                        )
                for hf in range(HF):
                    sl = slice(hf * 512, hf * 512 + 512)
                    nc.tensor.matmul(
                        o_ps[:, sl],
                        ones[0:1, 0:P],
                        b_row[0:1, sl],
                        start=False,
                        stop=True,
                    )
                ot = io_pool.tile([P, C], f32, name=f"ot{sc}", tag="io")
                nc.vector.tensor_copy(ot[:, :], o_ps[:, :])
                nc.sync.dma_start(out=out_d[sc * P : (sc + 1) * P, :], in_=ot[:, :])

    nc.compile()
    return nc


def _get_program():
    if "nc" not in _CACHE:
        _CACHE["nc"] = _build_program()
    return _CACHE["nc"]


def kernel(x, w_qkv, w_out, b_out):
    from concourse.bass_utils import run_bass_kernel_spmd

    nc = _get_program()
    x = np.ascontiguousarray(np.asarray(x, dtype=np.float32))
    w_qkv = np.ascontiguousarray(np.asarray(w_qkv, dtype=np.float32))
    w_out = np.ascontiguousarray(np.asarray(w_out, dtype=np.float32))
    b_row = np.ascontiguousarray(np.asarray(b_out, dtype=np.float32).reshape(1, C))
    in_maps = [
        {"x": x[i], "w_qkv": w_qkv, "w_out": w_out, "b_out": b_row} for i in range(B)
    ]
    res = run_bass_kernel_spmd(nc, in_maps, core_ids=list(range(B))).results
    return np.stack([res[i]["out"] for i in range(B)], axis=0)



# revision 4
# speedup vs baseline: 11163.4171x; 11163.4171x over previous
"""Multi-head attention block for Trainium2, 8-core data-parallel SPMD.

Computes, per batch element b (one NeuronCore each):
    qkv = x @ w_qkv ; q,k,v split into 16 heads of dim 64
    attn = softmax(q @ k^T / sqrt(64)) ; out = (attn @ v) @ w_out + b_out

Strategy (per core):
  - transpose x -> xT (c-major) via PE transposes
  - v computed in natural layout, written strided into v_aug tiles with a
    ones-column per head so the attention output matmul also produces the
    softmax row-sums for free
  - attention per head in transposed layout: s^T = kT^T @ qT on the PE,
    exp on ACT (1/8 scale folded in), o^T_aug accumulated over k chunks;
    softmax normalization deferred to o^T (DVE reciprocal + K=1 ones-matmul
    partition-broadcast)
  - the qT/kT projection of head pair t+1 is explicitly interleaved into
    the attention instruction stream of pair t (engines execute their
    streams in order, so overlap has to be emitted, not just scheduled)
  - out = o^T^T @ w_out + ones x b_out (bias added by the PE)
All matmul-feeding tiles are declared float32r (full PE rate; the producing
DVE/ACT/DMA instructions emit the FP32r rounding the BIR verifier requires).
"""

import sys

if "/opt/trn_rl_repo" not in sys.path:
    sys.path.insert(0, "/opt/trn_rl_repo")

import numpy as np

B = 8
N = 1024  # sequence length
C = 1024  # model dim
H = 16  # heads
D = 64  # head dim
P = 128  # partitions
NT = N // P  # seq chunks
CT = C // P  # channel chunks
HP = H // 2  # head pairs
SCALE = D ** -0.5
HF = C // 512  # free-dim halves per 1024 row

_CACHE = {}


def _build_program():
    from concourse import bacc, mybir
    import concourse.tile as tile
    from concourse.masks import make_identity

    f32 = mybir.dt.float32
    f32r = mybir.dt.float32r
    Exp = mybir.ActivationFunctionType.Exp

    nc = bacc.Bacc("TRN2", target_bir_lowering=False, debug=False)
    x_d = nc.declare_dram_parameter("x", [N, C], f32r, isOutput=False)
    wqkv_d = nc.declare_dram_parameter("w_qkv", [C, 3 * C], f32r, isOutput=False)
    wout_d = nc.declare_dram_parameter("w_out", [C, C], f32r, isOutput=False)
    bout_d = nc.declare_dram_parameter("b_out", [1, C], f32r, isOutput=False)
    out_d = nc.declare_dram_parameter("out", [N, C], f32, isOutput=True)

    with tile.TileContext(nc) as tc:
        with (
            tc.tile_pool(name="consts", bufs=1) as consts,
            tc.tile_pool(name="xTo", bufs=CT) as xT_pool,
            tc.tile_pool(name="vaug", bufs=NT) as vaug_pool,
            tc.tile_pool(name="psum", bufs=1, space="PSUM") as psum,
            tc.tile_pool(name="oTp", bufs=CT) as oT_pool,
            tc.tile_pool(name="io", bufs=3) as io_pool,
            tc.tile_pool(name="w", bufs=CT) as w_pool,
            tc.tile_pool(name="wqk", bufs=4) as wqk_pool,
            tc.tile_pool(name="pT", bufs=8) as pT_pool,
            tc.tile_pool(name="recip", bufs=1) as recip_pool,
            tc.tile_pool(name="bcs", bufs=1) as bcs_pool,
            tc.tile_pool(name="qkT", bufs=4) as qkT_pool,
        ):
            identity_f32 = consts.tile(
                [P, P], f32, name="identity_f32", tag="identity_f32"
            )
            make_identity(nc, identity_f32)
            # f32r transpose runs 1.5 PE cycles/row vs 2.0 for f32
            identity = consts.tile([P, P], f32r, name="identity", tag="identity")
            nc.vector.tensor_copy(identity[:, :], identity_f32[:, :])
            # memset can't emit f32r (ISA check) — stage in f32, round via copy
            ones_f32 = consts.tile([P, P], f32, name="ones_f32", tag="ones_f32")
            nc.vector.memset(ones_f32, 1.0)
            ones = consts.tile([1, P], f32r, name="ones", tag="ones")
            nc.vector.tensor_copy(ones[0:1, :], ones_f32[0:1, :])
            b_row = consts.tile([1, C], f32r, name="b_row", tag="b_row")
            nc.sync.dma_start(out=b_row[0:1, :], in_=bout_d[0:1, :])

            xT = [
                xT_pool.tile([P, N], f32r, name=f"xT{i}", tag="xTo") for i in range(CT)
            ]
            vaug = [
                vaug_pool.tile([P, H * (D + 1)], f32r, name=f"vaug{i}", tag="vaug")
                for i in range(NT)
            ]

            def mm_tile(name, tag, bufs):
                return psum.tile([P, C], f32, name=name, tag=tag, bufs=bufs)

            def half_tile(name, tag, bufs):
                return psum.tile([P, 512], f32, name=name, tag=tag, bufs=bufs)

            # ---------------- phase 0: transpose x into xT ----------------
            for si in range(NT):
                xin = io_pool.tile([P, C], f32r, name=f"xin{si}", tag="io")
                nc.sync.dma_start(out=xin[:, :], in_=x_d[si * P : (si + 1) * P, :])
                tr_ps = psum.tile([P, C], f32r, name=f"tr{si}", tag="mm", bufs=2)
                for ci in range(CT):
                    nc.tensor.transpose(
                        tr_ps[:, ci * P : (ci + 1) * P],
                        xin[:, ci * P : (ci + 1) * P],
                        identity,
                    )
                for ci in range(CT):
                    nc.vector.tensor_copy(
                        xT[ci][:, si * P : (si + 1) * P],
                        tr_ps[:, ci * P : (ci + 1) * P],
                    )

            # ---------- phase 1B: v (natural layout) -> v_aug ----------
            # 4 seq-chunks per pass (2x [P,C] from mm/acc tags + 2 halves in
            # the sT slots) -> w_v rows streamed only twice.
            for sc0 in range(0, NT, 4):
                scs = list(range(sc0, sc0 + 4))
                full = {scs[0]: mm_tile(f"vps{scs[0]}", "mm", 2),
                        scs[1]: mm_tile(f"vps{scs[1]}", "mm", 2),
                        scs[2]: mm_tile(f"vps{scs[2]}", "acc", 1)}
                sc3 = scs[3]
                halves = [
                    half_tile(f"vps{sc3}_0", "sT", 2),
                    half_tile(f"vps{sc3}_1", "sT", 2),
                ]
                for ci in range(CT):
                    wv = w_pool.tile([P, C], f32r, name=f"wv{sc0}_{ci}", tag="w")
                    nc.sync.dma_start(
                        out=wv[:, :],
                        in_=wqkv_d[ci * P : (ci + 1) * P, 2 * C : 3 * C],
                    )
                    st = dict(start=(ci == 0), stop=(ci == CT - 1))
                    for hf in range(HF):
                        sl = slice(hf * 512, hf * 512 + 512)
                        for sc in scs[:3]:
                            nc.tensor.matmul(
                                full[sc][:, sl],
                                xT[ci][:, sc * P : (sc + 1) * P],
                                wv[:, sl],
                                **st,
                            )
                        nc.tensor.matmul(
                            halves[hf][:, :],
                            xT[ci][:, sc3 * P : (sc3 + 1) * P],
                            wv[:, sl],
                            **st,
                        )
                for sc in scs[:3]:
                    va3 = vaug[sc].rearrange("p (h u) -> p h u", u=D + 1)
                    nc.vector.tensor_copy(
                        va3[:, :, D : D + 1],
                        ones_f32[:, 0:H].rearrange("p (h u) -> p h u", u=1),
                    )
                    nc.vector.tensor_copy(
                        va3[:, :, 0:D],
                        full[sc].rearrange("p (h u) -> p h u", u=D),
                    )
                va3 = vaug[sc3].rearrange("p (h u) -> p h u", u=D + 1)
                nc.vector.tensor_copy(
                    va3[:, :, D : D + 1],
                    ones_f32[:, 0:H].rearrange("p (h u) -> p h u", u=1),
                )
                for hf in range(HF):
                    nc.vector.tensor_copy(
                        va3[:, 8 * hf : 8 * hf + 8, 0:D],
                        halves[hf].rearrange("p (h u) -> p h u", u=D),
                    )

            # ---- interleaved: attention pair t || qT/kT projection pair t+1 ----
            def qkv_pair_steps(t, qTt, kTt, q_ps, k_ps):
                """Generator: one ci-step (2 weight DMAs + 4 matmuls) per next();
                finishes with the PSUM->SBUF copies."""
                for ci in range(CT):
                    wq = wqk_pool.tile([P, P], f32r, name=f"wq{t}_{ci}", tag="wqk")
                    nc.sync.dma_start(
                        out=wq[:, :],
                        in_=wqkv_d[ci * P : (ci + 1) * P, t * P : (t + 1) * P],
                    )
                    wk = wqk_pool.tile([P, P], f32r, name=f"wk{t}_{ci}", tag="wqk")
                    nc.sync.dma_start(
                        out=wk[:, :],
                        in_=wqkv_d[ci * P : (ci + 1) * P, C + t * P : C + (t + 1) * P],
                    )
                    st = dict(start=(ci == 0), stop=(ci == CT - 1))
                    for hf in range(HF):
                        sl = slice(hf * 512, hf * 512 + 512)
                        nc.tensor.matmul(q_ps[:, sl], wq[:, :], xT[ci][:, sl], **st)
                        nc.tensor.matmul(k_ps[:, sl], wk[:, :], xT[ci][:, sl], **st)
                    yield
                nc.vector.tensor_copy(qTt[:, :], q_ps[:, :])
                nc.vector.tensor_copy(kTt[:, :], k_ps[:, :])
                yield

            def new_pair_qkv(t):
                qTt = qkT_pool.tile([P, N], f32r, name=f"qT{t}", tag="qkT")
                kTt = qkT_pool.tile([P, N], f32r, name=f"kT{t}", tag="qkT")
                q_ps = mm_tile(f"qps{t}", "mm", 2)
                k_ps = mm_tile(f"kps{t}", "mm", 2)
                return qTt, kTt, qkv_pair_steps(t, qTt, kTt, q_ps, k_ps)

            oT = [
                oT_pool.tile([P, N], f32r, name=f"oT{i}", tag="oTp")
                for i in range(CT)
            ]

            # prologue: pair 0 projection emitted straight
            qT_cur, kT_cur, gen0 = new_pair_qkv(0)
            for _ in gen0:
                pass

            # w_out is prefetched one row-chunk per head pair (inside the
            # pair loop) so the DMAs spread across the attention region
            wos = []

            def prefetch_wo(ci):
                wo = w_pool.tile([P, C], f32r, name=f"wo{ci}", tag="w")
                nc.sync.dma_start(out=wo[:, :], in_=wout_d[ci * P : (ci + 1) * P, :])
                wos.append(wo)

            pending_norm = None
            for t in range(HP):
                prefetch_wo(t)
                if t + 1 < HP:
                    qT_nxt, kT_nxt, gen = new_pair_qkv(t + 1)
                else:
                    qT_nxt = kT_nxt = gen = None
                chunk_idx = 0
                NCH = NT * HF  # 16 chunks per head
                LAG = 4  # o^T matmuls trail s/exp by LAG chunks so the
                # previous head's normalize chain hides inside the stream
                for j in range(2):
                    h = 2 * t + j
                    row0 = D * j
                    acc = mm_tile(f"acc{h}", "acc", 1)

                    def ot_mm(c, acc=acc, h=h):
                        kc, hf = divmod(c, HF)
                        sl = slice(hf * 512, hf * 512 + 512)
                        nc.tensor.matmul(
                            acc[0 : D + 1, sl],
                            vaug[kc][:, h * (D + 1) : (h + 1) * (D + 1)],
                            pts[c][:, :],
                            start=(kc == 0),
                            stop=(kc == NT - 1),
                        )

                    pts = {}
                    for c in range(NCH):
                        kc, hf = divmod(c, HF)
                        sl = slice(hf * 512, hf * 512 + 512)
                        s_ps = half_tile(f"s{h}_{kc}_{hf}", "sT", 2)
                        nc.tensor.matmul(
                            s_ps[:, :],
                            kT_cur[row0 : row0 + D, kc * P : (kc + 1) * P],
                            qT_cur[row0 : row0 + D, sl],
                            start=True,
                            stop=True,
                        )
                        pt = pT_pool.tile(
                            [P, 512], f32r, name=f"pt{h}_{kc}_{hf}", tag="pT"
                        )
                        nc.scalar.activation(
                            out=pt[:, :], in_=s_ps[:, :], func=Exp, scale=SCALE
                        )
                        pts[c] = pt
                        if c == LAG - 2 and pending_norm is not None:
                            pending_norm()
                            pending_norm = None
                        if c >= LAG:
                            ot_mm(c - LAG)
                            del pts[c - LAG]
                        # sprinkle next pair's projection into the stream
                        if gen is not None and chunk_idx % 3 == 2:
                            next(gen, None)
                        chunk_idx += 1
                    for c in range(NCH - LAG, NCH):
                        ot_mm(c)

                    def normalize(h=h, row0=row0, t=t, acc=acc):
                        # o^T[d, q] *= 1 / rowsum[q].  The full-precision DVE
                        # reciprocal takes 6.5us on a [1, N] row and hard-blocks
                        # the strict-FIFO DVE queue (stalling the PE-feeding
                        # copies long enough to re-throttle HAM); the approx
                        # variant (~18 bits, plenty here) is ~5x faster, and the
                        # partition broadcast moves to the idle GpSimd engine so
                        # the PE stream never sees this chain at all.
                        # stage the rowsum row to a partition-0 SBUF tile with a
                        # regular copy first — the custom-DVE approx op is only
                        # known-good on base-partition-0 operands.
                        rs = recip_pool.tile([1, N], f32, name=f"rs{h}", tag="rsum")
                        nc.vector.tensor_copy(rs[0:1, :], acc[D : D + 1, :])
                        rc = recip_pool.tile([1, N], f32, name=f"rc{h}", tag="recip")
                        nc.vector.reciprocal_approx_fast(rc[0:1, :], rs[0:1, :])
                        bcs = bcs_pool.tile([D, N], f32, name=f"bcs{h}", tag="bcs")
                        nc.gpsimd.partition_broadcast(
                            bcs[0:D, :], rc[0:1, :], channels=D
                        )
                        nc.vector.tensor_mul(
                            oT[t][row0 : row0 + D, :],
                            acc[0:D, :],
                            bcs[0:D, :],
                        )

                    pending_norm = normalize
                if gen is not None:
                    for _ in gen:
                        pass
                qT_cur, kT_cur = qT_nxt, kT_nxt
            pending_norm()  # last head's normalize

            # ---------------- phase 3: out = o @ w_out + b ----------------
            for sc in range(NT):
                o_ps = mm_tile(f"ops{sc}", "mm", 2)
                for ci in range(CT):
                    for hf in range(HF):
                        sl = slice(hf * 512, hf * 512 + 512)
                        nc.tensor.matmul(
                            o_ps[:, sl],
                            oT[ci][:, sc * P : (sc + 1) * P],
                            wos[ci][:, sl],
                            start=(ci == 0),
                            stop=False,
                        )
                for hf in range(HF):
                    sl = slice(hf * 512, hf * 512 + 512)
                    nc.tensor.matmul(
                        o_ps[:, sl],
                        ones[0:1, 0:P],
                        b_row[0:1, sl],
                        start=False,
                        stop=True,
                    )
                ot = io_pool.tile([P, C], f32, name=f"ot{sc}", tag="io")
                nc.vector.tensor_copy(ot[:, :], o_ps[:, :])
                nc.sync.dma_start(out=out_d[sc * P : (sc + 1) * P, :], in_=ot[:, :])

    nc.compile()
    return nc


def _get_program():
    if "nc" not in _CACHE:
        _CACHE["nc"] = _build_program()
    return _CACHE["nc"]


def _in_maps(inputs):
    x = np.ascontiguousarray(np.asarray(inputs["x"], dtype=np.float32))
    w_qkv = np.ascontiguousarray(np.asarray(inputs["w_qkv"], dtype=np.float32))
    w_out = np.ascontiguousarray(np.asarray(inputs["w_out"], dtype=np.float32))
    b_row = np.ascontiguousarray(
        np.asarray(inputs["b_out"], dtype=np.float32).reshape(1, C)
    )
    return [
        {"x": x[i], "w_qkv": w_qkv, "w_out": w_out, "b_out": b_row} for i in range(B)
    ]


def kernel(x, w_qkv, w_out, b_out):
    from concourse.bass_utils import run_bass_kernel_spmd

    nc = _get_program()
    in_maps = _in_maps({"x": x, "w_qkv": w_qkv, "w_out": w_out, "b_out": b_out})
    res = run_bass_kernel_spmd(nc, in_maps, core_ids=list(range(B))).results
    return np.stack([res[i]["out"] for i in range(B)], axis=0)



# revision 7
# speedup vs baseline: 14274.1587x; 1.2787x over previous
"""Multi-head attention block for Trainium2, 8-core data-parallel SPMD.

Computes, per batch element b (one NeuronCore each):
    qkv = x @ w_qkv ; q,k,v split into 16 heads of dim 64
    attn = softmax(q @ k^T / sqrt(64)) ; out = (attn @ v) @ w_out + b_out

Design notes (v3, bf16):
  - All PE operands are bf16 (inputs are cast on the host): halves DMA
    and DVE traffic and gives LDWEIGHTS headroom to hide under matmuls.
    PSUM accumulation stays fp32.
  - The kernel is paced by the PE stream, which must stay *gapless*: the
    HAM clock gate only holds K=8/8 (2.4 GHz) while the PE never idles,
    and a single >3us bubble costs a 2x clock penalty for a long
    stretch.  Every off-PE chain (softmax normalize, projections'
    PSUM->SBUF casts) is arranged so the PE never waits on it.
  - Attention per head in transposed layout: s^T = kT^T @ qT, exp on ACT
    (one [128,1024] activation per (head, k-chunk) to amortize the
    ~350-cycle ACT fixed cost), o^T += vaug^T @ p^T with a ones column
    per head producing the softmax row-sums in the same matmuls.
  - AV matmuls trail the exp stream by LAG chunks (a global queue across
    head boundaries) so ACT latency never stalls the PE.
  - Softmax normalize is fully off the PE: the accumulator is staged to
    SBUF (freeing its PSUM bank for the next head), reciprocal via the
    fast approx custom-DVE op, partition-broadcast on GpSimd, final
    multiply on DVE.
  - PSUM budget (8 banks): sT [P,1024]x2 on the "mm" tag (4) + proj
    [P,512] (1) + acc [65,512]x3 rotation (3).  The "mm" tag is reused
    by transposes / v-proj / out-proj in the other phases.
  - q/k projections of pair t+1 are emitted interleaved into pair t's
    attention slots (one 512-wide accumulation step per slot), keeping
    the PE stream dense through the whole attention phase.
"""

import sys
from collections import deque

if "/opt/trn_rl_repo" not in sys.path:
    sys.path.insert(0, "/opt/trn_rl_repo")

import numpy as np

B = 8
N = 1024  # sequence length
C = 1024  # model dim
H = 16  # heads
D = 64  # head dim
P = 128  # partitions
NT = N // P  # seq chunks
CT = C // P  # channel chunks
HP = H // 2  # head pairs
SCALE = D ** -0.5
HF = C // 512  # 512-wide halves per 1024 row
LAG = 4  # AV matmuls trail the S/exp stream by this many k-chunks

_CACHE = {}


def _build_program():
    from concourse import bacc, mybir
    import concourse.tile as tile
    from concourse.masks import make_identity

    f32 = mybir.dt.float32
    bf16 = mybir.dt.bfloat16
    Exp = mybir.ActivationFunctionType.Exp

    nc = bacc.Bacc("TRN2", target_bir_lowering=False, debug=False)
    x_d = nc.declare_dram_parameter("x", [N, C], bf16, isOutput=False)
    wqkv_d = nc.declare_dram_parameter("w_qkv", [C, 3 * C], bf16, isOutput=False)
    wout_d = nc.declare_dram_parameter("w_out", [C, C], bf16, isOutput=False)
    bout_d = nc.declare_dram_parameter("b_out", [1, C], bf16, isOutput=False)
    out_d = nc.declare_dram_parameter("out", [N, C], f32, isOutput=True)

    with tile.TileContext(nc) as tc:
        with (
            tc.tile_pool(name="consts", bufs=1) as consts,
            tc.tile_pool(name="xTo", bufs=CT) as xT_pool,
            tc.tile_pool(name="vaug", bufs=NT) as vaug_pool,
            tc.tile_pool(name="psum", bufs=1, space="PSUM") as psum,
            tc.tile_pool(name="oTp", bufs=CT) as oT_pool,
            tc.tile_pool(name="io", bufs=3) as io_pool,
            tc.tile_pool(name="wv", bufs=CT) as wv_pool,
            tc.tile_pool(name="wo", bufs=CT) as wo_pool,
            tc.tile_pool(name="wqk", bufs=34) as wqk_pool,
            tc.tile_pool(name="qkT", bufs=4) as qkT_pool,
            tc.tile_pool(name="pT", bufs=LAG + 3) as pT_pool,
            tc.tile_pool(name="oTu", bufs=2) as oTu_pool,
            tc.tile_pool(name="rsum", bufs=2) as rs_pool,
            tc.tile_pool(name="recip", bufs=2) as rc_pool,
            tc.tile_pool(name="bcs", bufs=2) as bcs_pool,
        ):
            identity_f32 = consts.tile(
                [P, P], f32, name="identity_f32", tag="identity_f32"
            )
            make_identity(nc, identity_f32)
            identity = consts.tile([P, P], bf16, name="identity", tag="identity")
            nc.vector.tensor_copy(identity[:, :], identity_f32[:, :])
            ones_f32 = consts.tile([P, P], f32, name="ones_f32", tag="ones_f32")
            nc.vector.memset(ones_f32, 1.0)
            ones = consts.tile([1, P], bf16, name="ones", tag="ones")
            nc.vector.tensor_copy(ones[0:1, :], ones_f32[0:1, :])
            b_row = consts.tile([1, C], bf16, name="b_row", tag="b_row")
            nc.sync.dma_start(out=b_row[0:1, :], in_=bout_d[0:1, :])

            xT = [
                xT_pool.tile([P, N], bf16, name=f"xT{i}", tag="xTo") for i in range(CT)
            ]
            vaug = [
                vaug_pool.tile([P, H * (D + 1)], bf16, name=f"vaug{i}", tag="vaug")
                for i in range(NT)
            ]
            oT = [
                oT_pool.tile([P, N], bf16, name=f"oT{i}", tag="oTp") for i in range(CT)
            ]

            def mm_tile(name, dtype=f32):
                return psum.tile([P, N], dtype, name=name, tag="mm", bufs=2)

            # ---------------- phase 0: transpose x into xT ----------------
            for si in range(NT):
                xin = io_pool.tile([P, N], bf16, name=f"xin{si}", tag="io")
                nc.sync.dma_start(out=xin[:, :], in_=x_d[si * P : (si + 1) * P, :])
                tr_ps = mm_tile(f"tr{si}", bf16)
                for ci in range(CT):
                    nc.tensor.transpose(
                        tr_ps[:, ci * P : (ci + 1) * P],
                        xin[:, ci * P : (ci + 1) * P],
                        identity,
                    )
                for ci in range(CT):
                    nc.vector.tensor_copy(
                        xT[ci][:, si * P : (si + 1) * P],
                        tr_ps[:, ci * P : (ci + 1) * P],
                    )

            # ---- interleavable q/k projection generator (one pair) ----
            def pair_proj_gen(t, qTt, kTt):
                """Yields once per 512-wide accumulation step (32 total);
                prefetches all weight chunks at creation; finishes with the
                PSUM->SBUF casts emitted between sub-phases."""
                wtiles = {}
                for which, colbase in (("q", t * P), ("k", C + t * P)):
                    for ci in range(CT):
                        w = wqk_pool.tile(
                            [P, P], bf16, name=f"w{which}{t}_{ci}", tag="wqk"
                        )
                        nc.sync.dma_start(
                            out=w[:, :],
                            in_=wqkv_d[ci * P : (ci + 1) * P, colbase : colbase + P],
                        )
                        wtiles[(which, ci)] = w
                for which, dst in (("q", qTt), ("k", kTt)):
                    for sh in range(HF):
                        sl = slice(sh * 512, sh * 512 + 512)
                        ps = psum.tile(
                            [P, 512], f32, name=f"pj{t}{which}{sh}", tag="proj", bufs=1
                        )
                        for ci in range(CT):
                            nc.tensor.matmul(
                                ps[:, :],
                                wtiles[(which, ci)][:, :],
                                xT[ci][:, sl],
                                start=(ci == 0),
                                stop=(ci == CT - 1),
                            )
                            yield
                        nc.vector.tensor_copy(dst[:, sl], ps[:, :])

            def new_pair(t):
                qTt = qkT_pool.tile([P, N], bf16, name=f"qT{t}", tag="qkT")
                kTt = qkT_pool.tile([P, N], bf16, name=f"kT{t}", tag="qkT")
                return qTt, kTt, pair_proj_gen(t, qTt, kTt)

            # -------- phase 1: v projection (+ pair-0 q/k interleaved) --------
            wv = []
            for ci in range(CT):
                w = wv_pool.tile([P, N], bf16, name=f"wv{ci}", tag="wv")
                nc.sync.dma_start(
                    out=w[:, :], in_=wqkv_d[ci * P : (ci + 1) * P, 2 * C : 3 * C]
                )
                wv.append(w)

            qT_cur, kT_cur, gen0 = new_pair(0)
            for sc in range(NT):
                v_ps = mm_tile(f"vps{sc}")
                for ci in range(CT):
                    st = dict(start=(ci == 0), stop=(ci == CT - 1))
                    for hf in range(HF):
                        sl = slice(hf * 512, hf * 512 + 512)
                        nc.tensor.matmul(
                            v_ps[:, sl],
                            xT[ci][:, sc * P : (sc + 1) * P],
                            wv[ci][:, sl],
                            **st,
                        )
                    next(gen0, None)
                va3 = vaug[sc].rearrange("p (h u) -> p h u", u=D + 1)
                nc.vector.tensor_copy(
                    va3[:, :, D : D + 1],
                    ones_f32[:, 0:H].rearrange("p (h u) -> p h u", u=1),
                )
                nc.vector.tensor_copy(
                    va3[:, :, 0:D],
                    v_ps.rearrange("p (h u) -> p h u", u=D),
                )
            for _ in gen0:  # drain any remaining projection steps
                pass

            # ---------------- phase 2: attention ----------------
            wos = []

            def prefetch_wo(ci):
                wo = wo_pool.tile([P, N], bf16, name=f"wo{ci}", tag="wo")
                nc.sync.dma_start(out=wo[:, :], in_=wout_d[ci * P : (ci + 1) * P, :])
                wos.append(wo)

            def normalize(h, t, row0, accs):
                # stage the accumulator out of PSUM (frees the acc banks),
                # then 1/rowsum (approx) -> partition-broadcast -> multiply.
                oTu = oTu_pool.tile([D + 1, N], f32, name=f"oTu{h}", tag="oTu")
                rs = rs_pool.tile([1, N], f32, name=f"rs{h}", tag="rsum")
                for hf in range(HF):
                    sl = slice(hf * 512, hf * 512 + 512)
                    nc.vector.tensor_copy(oTu[0 : D + 1, sl], accs[hf][0 : D + 1, :])
                    nc.vector.tensor_copy(rs[0:1, sl], accs[hf][D : D + 1, :])
                rc = rc_pool.tile([1, N], f32, name=f"rc{h}", tag="recip")
                nc.vector.reciprocal_approx_fast(rc[0:1, :], rs[0:1, :])
                bcs = bcs_pool.tile([D, N], f32, name=f"bcs{h}", tag="bcs")
                nc.gpsimd.partition_broadcast(bcs[0:D, :], rc[0:1, :], channels=D)
                nc.vector.tensor_mul(
                    oT[t][row0 : row0 + D, :], oTu[0:D, :], bcs[0:D, :]
                )

            av_queue = deque()  # (emit_fn, head_done_fn | None)

            def drain_av(n):
                for _ in range(n):
                    if not av_queue:
                        return
                    emit, done = av_queue.popleft()
                    emit()
                    if done is not None:
                        done()

            for t in range(HP):
                prefetch_wo(t)
                if t + 1 < HP:
                    qT_nxt, kT_nxt, gen = new_pair(t + 1)
                else:
                    qT_nxt = kT_nxt = gen = None
                for j in range(2):
                    h = 2 * t + j
                    row0 = D * j
                    accs = [
                        psum.tile([P, 512], f32, name=f"acc{h}_{hf}", tag="acc", bufs=3)
                        for hf in range(HF)
                    ]

                    def make_av(kc, pt, accs=accs, h=h):
                        def emit():
                            for hf in range(HF):
                                sl = slice(hf * 512, hf * 512 + 512)
                                nc.tensor.matmul(
                                    accs[hf][0 : D + 1, :],
                                    vaug[kc][:, h * (D + 1) : (h + 1) * (D + 1)],
                                    pt[:, sl],
                                    start=(kc == 0),
                                    stop=(kc == NT - 1),
                                )

                        return emit

                    done_fn = (
                        lambda h=h, t=t, row0=row0, accs=accs: normalize(
                            h, t, row0, accs
                        )
                    )
                    for kc in range(NT):
                        sT = mm_tile(f"s{h}_{kc}")
                        for hf in range(HF):
                            sl = slice(hf * 512, hf * 512 + 512)
                            nc.tensor.matmul(
                                sT[:, sl],
                                kT_cur[row0 : row0 + D, kc * P : (kc + 1) * P],
                                qT_cur[row0 : row0 + D, sl],
                                start=True,
                                stop=True,
                            )
                        pt = pT_pool.tile([P, N], bf16, name=f"pt{h}_{kc}", tag="pT")
                        nc.scalar.activation(
                            out=pt[:, :], in_=sT[:, :], func=Exp, scale=SCALE
                        )
                        av_queue.append(
                            (make_av(kc, pt), done_fn if kc == NT - 1 else None)
                        )
                        if len(av_queue) > LAG:
                            drain_av(len(av_queue) - LAG)
                        if gen is not None:
                            # 32 projection steps per pair over 16 slots
                            next(gen, None)
                            next(gen, None)
                if gen is not None:
                    for _ in gen:
                        pass
                qT_cur, kT_cur = qT_nxt, kT_nxt
            drain_av(len(av_queue))  # last head's AV tail + its normalize

            # ---------------- phase 3: out = o @ w_out + b ----------------
            for sc in range(NT):
                o_ps = mm_tile(f"ops{sc}")
                for ci in range(CT):
                    for hf in range(HF):
                        sl = slice(hf * 512, hf * 512 + 512)
                        nc.tensor.matmul(
                            o_ps[:, sl],
                            oT[ci][:, sc * P : (sc + 1) * P],
                            wos[ci][:, sl],
                            start=(ci == 0),
                            stop=False,
                        )
                for hf in range(HF):
                    sl = slice(hf * 512, hf * 512 + 512)
                    nc.tensor.matmul(
                        o_ps[:, sl],
                        ones[0:1, 0:P],
                        b_row[0:1, sl],
                        start=False,
                        stop=True,
                    )
                ot = io_pool.tile([P, C], f32, name=f"ot{sc}", tag="ot")
                nc.vector.tensor_copy(ot[:, :], o_ps[:, :])
                nc.sync.dma_start(out=out_d[sc * P : (sc + 1) * P, :], in_=ot[:, :])

    nc.compile()
    return nc


def _get_program():
    if "nc" not in _CACHE:
        _CACHE["nc"] = _build_program()
    return _CACHE["nc"]


def _bf16(a):
    import ml_dtypes

    return np.ascontiguousarray(np.asarray(a, dtype=np.float32)).astype(
        ml_dtypes.bfloat16
    )


def _in_maps(inputs):
    x = _bf16(inputs["x"])
    w_qkv = _bf16(inputs["w_qkv"])
    w_out = _bf16(inputs["w_out"])
    b_row = _bf16(np.asarray(inputs["b_out"]).reshape(1, C))
    return [
        {"x": x[i], "w_qkv": w_qkv, "w_out": w_out, "b_out": b_row} for i in range(B)
    ]


def kernel(x, w_qkv, w_out, b_out):
    from concourse.bass_utils import run_bass_kernel_spmd

    nc = _get_program()
    in_maps = _in_maps({"x": x, "w_qkv": w_qkv, "w_out": w_out, "b_out": b_out})
    res = run_bass_kernel_spmd(nc, in_maps, core_ids=list(range(B))).results
    return np.stack([res[i]["out"] for i in range(B)], axis=0)


# revision 18
# speedup vs baseline: 17495.4303x; 1.2257x over previous
"""Multi-head attention block for Trainium2, 8-core data-parallel SPMD.

Computes, per batch element b (one NeuronCore each):
    qkv = x @ w_qkv ; q,k,v split into 16 heads of dim 64
    attn = softmax(q @ k^T / sqrt(64)) ; out = (attn @ v) @ w_out + b_out

Design notes (v3, bf16):
  - All PE operands are bf16 (inputs are cast on the host): halves DMA
    and DVE traffic and gives LDWEIGHTS headroom to hide under matmuls.
    PSUM accumulation stays fp32.
  - The kernel is paced by the PE stream, which must stay *gapless*: the
    HAM clock gate only holds K=8/8 (2.4 GHz) while the PE never idles,
    and a single >3us bubble costs a 2x clock penalty for a long
    stretch.  Every off-PE chain (softmax normalize, projections'
    PSUM->SBUF casts) is arranged so the PE never waits on it.
  - Attention per head in transposed layout: s^T = kT^T @ qT, exp on ACT
    (one [128,1024] activation per (head, k-chunk) to amortize the
    ~350-cycle ACT fixed cost), o^T += vaug^T @ p^T with a ones column
    per head producing the softmax row-sums in the same matmuls.
  - AV matmuls trail the exp stream by LAG chunks (a global queue across
    head boundaries) so ACT latency never stalls the PE.
  - Softmax normalize is fully off the PE: the accumulator is staged to
    SBUF (freeing its PSUM bank for the next head), reciprocal via the
    fast approx custom-DVE op, partition-broadcast on GpSimd, final
    multiply on DVE.
  - PSUM budget (8 banks): sT [P,1024]x2 on the "mm" tag (4) + proj
    [P,512] (1) + acc [65,512]x3 rotation (3).  The "mm" tag is reused
    by transposes / v-proj / out-proj in the other phases.
  - q/k projections of pair t+1 are emitted interleaved into pair t's
    attention slots (one 512-wide accumulation step per slot), keeping
    the PE stream dense through the whole attention phase.
"""

import sys
from collections import deque

if "/opt/trn_rl_repo" not in sys.path:
    sys.path.insert(0, "/opt/trn_rl_repo")

import numpy as np

B = 8
N = 1024  # sequence length
C = 1024  # model dim
H = 16  # heads
D = 64  # head dim
P = 128  # partitions
NT = N // P  # seq chunks
CT = C // P  # channel chunks
HP = H // 2  # head pairs
SCALE = D ** -0.5
HF = C // 512  # 512-wide halves per 1024 row
LAG = 4  # AV matmuls trail the S/exp stream by this many k-chunks

_CACHE = {}


def _build_program():
    from concourse import bacc, mybir
    import concourse.tile as tile
    from concourse.masks import make_identity

    f32 = mybir.dt.float32
    bf16 = mybir.dt.bfloat16
    Exp = mybir.ActivationFunctionType.Exp

    nc = bacc.Bacc("TRN2", target_bir_lowering=False, debug=False)
    x_d = nc.declare_dram_parameter("x", [N, C], bf16, isOutput=False)
    wqkv_d = nc.declare_dram_parameter("w_qkv", [C, 3 * C], bf16, isOutput=False)
    wout_d = nc.declare_dram_parameter("w_out", [C, C], bf16, isOutput=False)
    bout_d = nc.declare_dram_parameter("b_out", [1, C], bf16, isOutput=False)
    out_d = nc.declare_dram_parameter("out", [N, C], f32, isOutput=True)

    with tile.TileContext(nc) as tc:
        with (
            tc.tile_pool(name="consts", bufs=1) as consts,
            tc.tile_pool(name="xTo", bufs=CT) as xT_pool,
            tc.tile_pool(name="vaug", bufs=NT) as vaug_pool,
            tc.tile_pool(name="psum", bufs=1, space="PSUM") as psum,
            tc.tile_pool(name="oTp", bufs=CT) as oT_pool,
            tc.tile_pool(name="io", bufs=3) as io_pool,
            tc.tile_pool(name="xin", bufs=NT) as xin_pool,
            tc.tile_pool(name="wv", bufs=CT) as wv_pool,
            tc.tile_pool(name="wo", bufs=CT) as wo_pool,
            tc.tile_pool(name="wqk", bufs=4) as wqk_pool,
            tc.tile_pool(name="qkT", bufs=4) as qkT_pool,
            tc.tile_pool(name="pT", bufs=LAG + 2) as pT_pool,
            tc.tile_pool(name="oTu", bufs=2) as oTu_pool,
            tc.tile_pool(name="rsum", bufs=2) as rs_pool,
            tc.tile_pool(name="recip", bufs=2) as rc_pool,
            tc.tile_pool(name="bcs", bufs=2) as bcs_pool,
        ):
            identity_f32 = consts.tile(
                [P, P], f32, name="identity_f32", tag="identity_f32"
            )
            make_identity(nc, identity_f32)
            identity = consts.tile([P, P], bf16, name="identity", tag="identity")
            nc.vector.tensor_copy(identity[:, :], identity_f32[:, :])
            ones_f32 = consts.tile([P, P], f32, name="ones_f32", tag="ones_f32")
            nc.vector.memset(ones_f32, 1.0)
            b_row = consts.tile([1, C], bf16, name="b_row", tag="b_row")
            nc.sync.dma_start(out=b_row[0:1, :], in_=bout_d[0:1, :])
            # bias broadcast to all partitions once; phase 3 then adds it on
            # DVE in the PSUM->SBUF move instead of 16 PE matmuls
            b_f32 = consts.tile([1, C], f32, name="b_f32", tag="b_f32")
            nc.vector.tensor_copy(b_f32[0:1, :], b_row[0:1, :])
            bias_bc = consts.tile([P, C], f32, name="bias_bc", tag="bias_bc")
            nc.gpsimd.partition_broadcast(bias_bc[:, :], b_f32[0:1, :], channels=P)

            xT = [
                xT_pool.tile([P, N], bf16, name=f"xT{i}", tag="xTo") for i in range(CT)
            ]
            vaug = [
                vaug_pool.tile([P, H * (D + 1)], bf16, name=f"vaug{i}", tag="vaug")
                for i in range(NT)
            ]
            oT = [
                oT_pool.tile([P, N], bf16, name=f"oT{i}", tag="oTp") for i in range(CT)
            ]

            def mm_tile(name, dtype=f32):
                return psum.tile([P, N], dtype, name=name, tag="mm", bufs=2)

            # ---------------- phase 0: transpose x into xT ----------------
            xins = []
            for si in range(NT):
                xin = xin_pool.tile([P, N], bf16, name=f"xin{si}", tag="xin")
                nc.sync.dma_start(out=xin[:, :], in_=x_d[si * P : (si + 1) * P, :])
                xins.append(xin)
            for si in range(NT):
                xin = xins[si]
                tr_ps = mm_tile(f"tr{si}", bf16)
                for ci in range(CT):
                    nc.tensor.transpose(
                        tr_ps[:, ci * P : (ci + 1) * P],
                        xin[:, ci * P : (ci + 1) * P],
                        identity,
                    )
                for ci in range(CT):
                    nc.vector.tensor_copy(
                        xT[ci][:, si * P : (si + 1) * P],
                        tr_ps[:, ci * P : (ci + 1) * P],
                    )

            # ---- interleavable q/k projection generator (one pair) ----
            def pair_proj_gen(t, qTt, kTt):
                """Yields once per 512-wide accumulation step (32 total);
                prefetches all weight chunks at creation; finishes with the
                PSUM->SBUF casts emitted between sub-phases."""
                wtiles = {}
                for which, colbase in (("q", t * P), ("k", C + t * P)):
                    # all 8 contraction chunks of this weight column block in
                    # one strided DMA (row ci*128+p, col c -> [p, ci, c])
                    w = wqk_pool.tile([P, CT * P], bf16, name=f"w{which}{t}", tag="wqk")
                    nc.sync.dma_start(
                        out=w.rearrange("p (ci c) -> p ci c", c=P),
                        in_=wqkv_d[:, colbase : colbase + P].rearrange(
                            "(ci p) c -> p ci c", p=P
                        ),
                    )
                    wtiles[which] = w
                for which, dst in (("q", qTt), ("k", kTt)):
                    for sh in range(HF):
                        sl = slice(sh * 512, sh * 512 + 512)
                        ps = psum.tile(
                            [P, 512], f32, name=f"pj{t}{which}{sh}", tag="proj", bufs=2
                        )
                        for ci in range(CT):
                            nc.tensor.matmul(
                                ps[:, :],
                                wtiles[which][:, ci * P : (ci + 1) * P],
                                xT[ci][:, sl],
                                start=(ci == 0),
                                stop=(ci == CT - 1),
                            )
                            yield
                        nc.vector.tensor_copy(dst[:, sl], ps[:, :])

            def new_pair(t):
                qTt = qkT_pool.tile([P, N], bf16, name=f"qT{t}", tag="qkT")
                kTt = qkT_pool.tile([P, N], bf16, name=f"kT{t}", tag="qkT")
                return qTt, kTt, pair_proj_gen(t, qTt, kTt)

            # -------- phase 1: v projection (+ pair-0 q/k interleaved) --------
            wv = []
            for ci in range(CT):
                w = wv_pool.tile([P, N], bf16, name=f"wv{ci}", tag="wv")
                nc.sync.dma_start(
                    out=w[:, :], in_=wqkv_d[ci * P : (ci + 1) * P, 2 * C : 3 * C]
                )
                wv.append(w)

            qT_cur, kT_cur, gen0 = new_pair(0)
            for sc in range(NT):
                v_ps = mm_tile(f"vps{sc}")
                for ci in range(CT):
                    st = dict(start=(ci == 0), stop=(ci == CT - 1))
                    for hf in range(HF):
                        sl = slice(hf * 512, hf * 512 + 512)
                        nc.tensor.matmul(
                            v_ps[:, sl],
                            xT[ci][:, sc * P : (sc + 1) * P],
                            wv[ci][:, sl],
                            **st,
                        )
                    next(gen0, None)
                va3 = vaug[sc].rearrange("p (h u) -> p h u", u=D + 1)
                nc.vector.tensor_copy(
                    va3[:, :, D : D + 1],
                    ones_f32[:, 0:H].rearrange("p (h u) -> p h u", u=1),
                )
                nc.vector.tensor_copy(
                    va3[:, :, 0:D],
                    v_ps.rearrange("p (h u) -> p h u", u=D),
                )
            for _ in gen0:  # drain any remaining projection steps
                pass

            # ---------------- phase 2: attention ----------------
            wos = []

            def prefetch_wo(ci):
                wo = wo_pool.tile([P, N], bf16, name=f"wo{ci}", tag="wo")
                nc.sync.dma_start(out=wo[:, :], in_=wout_d[ci * P : (ci + 1) * P, :])
                wos.append(wo)

            def normalize(h, t, row0, accs):
                # stage the accumulator out of PSUM (frees the acc banks),
                # then 1/rowsum (approx) -> partition-broadcast -> multiply.
                oTu = oTu_pool.tile([D + 1, N], f32, name=f"oTu{h}", tag="oTu")
                rs = rs_pool.tile([1, N], f32, name=f"rs{h}", tag="rsum")
                for hf in range(HF):
                    sl = slice(hf * 512, hf * 512 + 512)
                    nc.vector.tensor_copy(oTu[0 : D + 1, sl], accs[hf][0 : D + 1, :])
                    nc.vector.tensor_copy(rs[0:1, sl], accs[hf][D : D + 1, :])
                rc = rc_pool.tile([1, N], f32, name=f"rc{h}", tag="recip")
                nc.vector.reciprocal_approx_fast(rc[0:1, :], rs[0:1, :])
                bcs = bcs_pool.tile([D, N], f32, name=f"bcs{h}", tag="bcs")
                nc.gpsimd.partition_broadcast(bcs[0:D, :], rc[0:1, :], channels=D)
                nc.vector.tensor_mul(
                    oT[t][row0 : row0 + D, :], oTu[0:D, :], bcs[0:D, :]
                )

            av_queue = deque()  # (emit_fn, head_done_fn | None)

            def drain_av(n):
                for _ in range(n):
                    if not av_queue:
                        return
                    emit, done = av_queue.popleft()
                    emit()
                    if done is not None:
                        done()

            for t in range(HP):
                prefetch_wo(t)
                if t + 1 < HP:
                    qT_nxt, kT_nxt, gen = new_pair(t + 1)
                else:
                    qT_nxt = kT_nxt = gen = None
                for j in range(2):
                    h = 2 * t + j
                    row0 = D * j
                    accs = [
                        psum.tile([P, 512], f32, name=f"acc{h}_{hf}", tag="acc", bufs=2)
                        for hf in range(HF)
                    ]

                    def make_av(kc, pt, accs=accs, h=h):
                        def emit():
                            for hf in range(HF):
                                sl = slice(hf * 512, hf * 512 + 512)
                                nc.tensor.matmul(
                                    accs[hf][0 : D + 1, :],
                                    vaug[kc][:, h * (D + 1) : (h + 1) * (D + 1)],
                                    pt[:, sl],
                                    start=(kc == 0),
                                    stop=(kc == NT - 1),
                                )

                        return emit

                    done_fn = (
                        lambda h=h, t=t, row0=row0, accs=accs: normalize(
                            h, t, row0, accs
                        )
                    )
                    for kc in range(NT):
                        sT = mm_tile(f"s{h}_{kc}")
                        for hf in range(HF):
                            sl = slice(hf * 512, hf * 512 + 512)
                            nc.tensor.matmul(
                                sT[:, sl],
                                kT_cur[row0 : row0 + D, kc * P : (kc + 1) * P],
                                qT_cur[row0 : row0 + D, sl],
                                start=True,
                                stop=True,
                            )
                        pt = pT_pool.tile([P, N], bf16, name=f"pt{h}_{kc}", tag="pT")
                        for hf in range(HF):
                            sl = slice(hf * 512, hf * 512 + 512)
                            nc.scalar.activation(
                                out=pt[:, sl], in_=sT[:, sl], func=Exp, scale=SCALE
                            )
                        av_queue.append(
                            (make_av(kc, pt), done_fn if kc == NT - 1 else None)
                        )
                        if len(av_queue) > LAG:
                            drain_av(len(av_queue) - LAG)
                        if gen is not None:
                            # 32 projection steps per pair over 16 slots
                            next(gen, None)
                            next(gen, None)
                if gen is not None:
                    for _ in gen:
                        pass
                qT_cur, kT_cur = qT_nxt, kT_nxt
            drain_av(len(av_queue))  # last head's AV tail + its normalize

            # ---------------- phase 3: out = o @ w_out + b ----------------
            for sc in range(NT):
                o_ps = mm_tile(f"ops{sc}")
                for ci in range(CT):
                    for hf in range(HF):
                        sl = slice(hf * 512, hf * 512 + 512)
                        nc.tensor.matmul(
                            o_ps[:, sl],
                            oT[ci][:, sc * P : (sc + 1) * P],
                            wos[ci][:, sl],
                            start=(ci == 0),
                            stop=(ci == CT - 1),
                        )
                ot = io_pool.tile([P, C], f32, name=f"ot{sc}", tag="ot")
                nc.vector.tensor_add(ot[:, :], o_ps[:, :], bias_bc[:, :])
                nc.sync.dma_start(out=out_d[sc * P : (sc + 1) * P, :], in_=ot[:, :])

    nc.compile()
    return nc


def _get_program():
    if "nc" not in _CACHE:
        _CACHE["nc"] = _build_program()
    return _CACHE["nc"]


def _bf16(a):
    import ml_dtypes

    return np.ascontiguousarray(np.asarray(a, dtype=np.float32)).astype(
        ml_dtypes.bfloat16
    )


def _in_maps(inputs):
    x = _bf16(inputs["x"])
    w_qkv = _bf16(inputs["w_qkv"])
    w_out = _bf16(inputs["w_out"])
    b_row = _bf16(np.asarray(inputs["b_out"]).reshape(1, C))
    return [
        {"x": x[i], "w_qkv": w_qkv, "w_out": w_out, "b_out": b_row} for i in range(B)
    ]


def kernel(x, w_qkv, w_out, b_out):
    from concourse.bass_utils import run_bass_kernel_spmd

    nc = _get_program()
    in_maps = _in_maps({"x": x, "w_qkv": w_qkv, "w_out": w_out, "b_out": b_out})
    res = run_bass_kernel_spmd(nc, in_maps, core_ids=list(range(B))).results
    return np.stack([res[i]["out"] for i in range(B)], axis=0)
